# revision 2
# baseline (speedup 1.0000x reference)
"""BinaryConnect 3x3 SAME conv (NHWC, 32x112x112x128 -> 32x112x112x256) on 8 trn2 cores.

Strategy (data-parallel, 4 images per core):
  - Host: binarize kernel to +/-1 fp16 (exact), cast x to fp16, transpose to
    channel-major [cin, n, hp, wp] with 1-px zero halo, rows padded to 114 and
    one zero tail row (115 rows total), flattened per image.
  - Device: out[cout_half, fo] accumulates 9 matmuls (one per 3x3 tap) in PSUM:
    lhsT = wb_tap [cin=128, cout_half=128] (stationary),
    rhs  = x_flat[cin=128, fo + dh*114 + dw : +S] (moving, S=456),
    fp16 in, fp32 PSUM accumulate. Output written channel-major, un-transposed
    on host. Padded output columns (w=112,113) are garbage and stripped on host.
"""

import os

import numpy as np

import concourse.bass as bass
import concourse.mybir as mybir
import concourse.tile as tile
from concourse import bacc
from concourse.bass_utils import run_bass_kernel_spmd

N_CORES = 8
NPC = 4            # images per core
H = 112
WP = 114           # padded row width
HP = 115           # 1 top pad + 112 rows + 1 bottom pad + 1 zero tail row
XF = HP * WP       # 13110 flat padded-input positions per image
FO = H * WP        # 12768 flat padded-output positions per image
S = 456            # matmul free dim (4*114, divides FO; <=512 fp32 PSUM bank)
TS = FO // S       # 28 spatial tiles per image
CI = 128
CO = 256

_nc_cache = None
LAST_RESULT = None


def _build():
    nc = bacc.Bacc(
        "TRN2",
        target_bir_lowering=False,
        debug=False,
        num_devices=N_CORES,
    )
    x_d = nc.dram_tensor("xp", [CI, NPC, XF], mybir.dt.float16, kind="ExternalInput")
    w_d = nc.dram_tensor("wt", [CI, 9 * CO], mybir.dt.float16, kind="ExternalInput")
    o_d = nc.dram_tensor(
        "out_cm", [CO, NPC, FO], mybir.dt.float32, kind="ExternalOutput"
    )
    offs = [dh * WP + dw for dh in range(3) for dw in range(3)]
    with tile.TileContext(nc) as tc:
        with (
            tc.tile_pool(name="xpool", bufs=1) as xpool,
            tc.tile_pool(name="wpool", bufs=1) as wpool,
            tc.tile_pool(name="psum", bufs=8, space=bass.MemorySpace.PSUM) as psum,
            tc.tile_pool(name="opool", bufs=8) as opool,
        ):
            wt_s = wpool.tile([CI, 9 * CO], mybir.dt.float16)
            nc.sync.dma_start(wt_s[:], w_d[:, :])
            # Band-split the input so the first matmuls only gate on ~880KB:
            # 4 bands of 28 output rows per image; each band loads 31 input
            # rows (halo) = 3534 flat positions.
            BROWS = 28            # output rows per band
            NB = H // BROWS       # 4 bands per image
            BS = BROWS * WP       # 3192 output flat positions per band
            BIN = (BROWS + 3) * WP  # 3534 input flat positions incl. halo
            TSB = BS // S         # 7 spatial tiles per band
            xs = {}
            for n in range(NPC):
                for b in range(NB):
                    xt = xpool.tile(
                        [CI, BIN], mybir.dt.float16, tag=f"x{n}_{b}", name=f"x{n}_{b}"
                    )
                    nc.sync.dma_start(xt[:], x_d[:, n, b * BS : b * BS + BIN])
                    xs[n, b] = xt
            for n in range(NPC):
                for b in range(NB):
                    for st in range(TSB):
                        s0 = b * BS + st * S   # image-relative output offset
                        r0 = st * S            # band-relative offset
                        for half in range(2):
                            ps = psum.tile([128, S], mybir.dt.float32, name="ps")
                            for t in range(9):
                                w0 = t * CO + half * 128
                                nc.tensor.matmul(
                                    ps[:],
                                    wt_s[:, w0 : w0 + 128],
                                    xs[n, b][:, r0 + offs[t] : r0 + offs[t] + S],
                                    start=(t == 0),
                                    stop=(t == 8),
                                )
                            ot = opool.tile([128, S], mybir.dt.float32, name="ot")
                            nc.vector.tensor_copy(ot[:], ps[:])
                            nc.sync.dma_start(
                                o_d[half * 128 : half * 128 + 128, n, s0 : s0 + S],
                                ot[:],
                            )
    nc.compile()
    return nc


def _get_nc():
    global _nc_cache
    if _nc_cache is None:
        _nc_cache = _build()
    return _nc_cache


def kernel(x, kernel):
    global LAST_RESULT
    x = np.asarray(x)
    k = np.asarray(kernel)

    wb = np.where(k >= 0, np.float16(1), np.float16(-1))  # [3,3,128,256]
    wt = np.ascontiguousarray(wb.transpose(2, 0, 1, 3).reshape(CI, 9 * CO))

    x16 = x.astype(np.float16)  # [32,112,112,128]
    in_maps = []
    for c in range(N_CORES):
        xp = np.zeros((CI, NPC, HP, WP), np.float16)
        xp[:, :, 1:113, 1:113] = x16[c * NPC : (c + 1) * NPC].transpose(3, 0, 1, 2)
        in_maps.append({"xp": xp.reshape(CI, NPC, XF), "wt": wt})

    nc = _get_nc()
    trace = os.environ.get("BCONV_TRACE", "0") == "1"
    res = run_bass_kernel_spmd(
        nc, in_maps, core_ids=list(range(N_CORES)), trace=trace
    )
    LAST_RESULT = res

    out = np.empty((32, H, H, CO), np.float32)
    for c in range(N_CORES):
        o = res.results[c]["out_cm"]  # [256, 4, 12768]
        o = o.reshape(CO, NPC, H, WP)[:, :, :, :112]
        out[c * NPC : (c + 1) * NPC] = o.transpose(1, 2, 3, 0)
    return out


# revision 4
# speedup vs baseline: 1.0270x; 1.0270x over previous
"""BinaryConnect 3x3 SAME conv (NHWC, 32x112x112x128 -> 32x112x112x256) on 8 trn2 cores.

Strategy (data-parallel, 4 images per core):
  - Host: binarize kernel to +/-1 fp16 (exact), cast x to fp16, transpose to
    channel-major [cin, n, hp, wp] with 1-px zero halo, rows padded to 114 and
    one zero tail row (115 rows total), flattened per image.
  - Device: out[cout_half, fo] accumulates 9 matmuls (one per 3x3 tap) in PSUM:
    lhsT = wb_tap [cin=128, cout_half=128] (stationary),
    rhs  = x_flat[cin=128, fo + dh*114 + dw : +S] (moving, S=456),
    fp16 in, fp32 PSUM accumulate. Output written channel-major, un-transposed
    on host. Padded output columns (w=112,113) are garbage and stripped on host.
"""

import os

import numpy as np

import concourse.bass as bass
import concourse.mybir as mybir
import concourse.tile as tile
from concourse import bacc
from concourse.bass_utils import run_bass_kernel_spmd

N_CORES = 8
NPC = 4            # images per core
H = 112
WP = 114           # padded row width
HP = 115           # 1 top pad + 112 rows + 1 bottom pad + 1 zero tail row
XF = HP * WP       # 13110 flat padded-input positions per image
FO = H * WP        # 12768 flat padded-output positions per image
S = 456            # matmul free dim (4*114, divides FO; <=512 fp32 PSUM bank)
TS = FO // S       # 28 spatial tiles per image
CI = 128
CO = 256

_nc_cache = None
LAST_RESULT = None


def _build():
    nc = bacc.Bacc(
        "TRN2",
        target_bir_lowering=False,
        debug=False,
        num_devices=N_CORES,
    )
    x_d = nc.dram_tensor("xp", [CI, NPC, XF], mybir.dt.float16, kind="ExternalInput")
    w_d = nc.dram_tensor("wt", [CI, 9 * CO], mybir.dt.float16, kind="ExternalInput")
    o_d = nc.dram_tensor(
        "out_cm", [CO, NPC, FO], mybir.dt.float32, kind="ExternalOutput"
    )
    offs = [dh * WP + dw for dh in range(3) for dw in range(3)]
    with tile.TileContext(nc) as tc:
        with (
            tc.tile_pool(name="xpool", bufs=1) as xpool,
            tc.tile_pool(name="wpool", bufs=1) as wpool,
            tc.tile_pool(name="psum", bufs=8, space=bass.MemorySpace.PSUM) as psum,
            tc.tile_pool(name="opool", bufs=12) as opool,
        ):
            wt_s = wpool.tile([CI, 9 * CO], mybir.dt.float16)
            nc.sync.dma_start(wt_s[:], w_d[:, :])
            # Band-split the input so the first matmuls only gate on ~880KB:
            # 4 bands of 28 output rows per image; each band loads 31 input
            # rows (halo) = 3534 flat positions.
            BROWS = 28            # output rows per band
            NB = H // BROWS       # 4 bands per image
            BS = BROWS * WP       # 3192 output flat positions per band
            BIN = (BROWS + 3) * WP  # 3534 input flat positions incl. halo
            TSB = BS // S         # 7 spatial tiles per band
            xs = {}
            for n in range(NPC):
                for b in range(NB):
                    xt = xpool.tile(
                        [CI, BIN], mybir.dt.float16, tag=f"x{n}_{b}", name=f"x{n}_{b}"
                    )
                    nc.sync.dma_start(xt[:], x_d[:, n, b * BS : b * BS + BIN])
                    xs[n, b] = xt
            for n in range(NPC):
                for b in range(NB):
                    for st in range(TSB):
                        s0 = b * BS + st * S   # image-relative output offset
                        r0 = st * S            # band-relative offset
                        for half in range(2):
                            ps = psum.tile([128, S], mybir.dt.float32, name="ps")
                            for t in range(9):
                                w0 = t * CO + half * 128
                                nc.tensor.matmul(
                                    ps[:],
                                    wt_s[:, w0 : w0 + 128],
                                    xs[n, b][:, r0 + offs[t] : r0 + offs[t] + S],
                                    start=(t == 0),
                                    stop=(t == 8),
                                )
                            ot = opool.tile([128, S], mybir.dt.float32, name="ot")
                            nc.vector.tensor_copy(ot[:], ps[:])
                            # ACT's HWDGE ring — keeps output DMAs off the
                            # sync ring so they don't queue behind input DMAs.
                            nc.scalar.dma_start(
                                o_d[half * 128 : half * 128 + 128, n, s0 : s0 + S],
                                ot[:],
                            )
    nc.compile()
    return nc


def _get_nc():
    global _nc_cache
    if _nc_cache is None:
        _nc_cache = _build()
    return _nc_cache


def kernel(x, kernel):
    global LAST_RESULT
    x = np.asarray(x)
    k = np.asarray(kernel)

    wb = np.where(k >= 0, np.float16(1), np.float16(-1))  # [3,3,128,256]
    wt = np.ascontiguousarray(wb.transpose(2, 0, 1, 3).reshape(CI, 9 * CO))

    x16 = x.astype(np.float16)  # [32,112,112,128]
    in_maps = []
    for c in range(N_CORES):
        xp = np.zeros((CI, NPC, HP, WP), np.float16)
        xp[:, :, 1:113, 1:113] = x16[c * NPC : (c + 1) * NPC].transpose(3, 0, 1, 2)
        in_maps.append({"xp": xp.reshape(CI, NPC, XF), "wt": wt})

    nc = _get_nc()
    trace = os.environ.get("BCONV_TRACE", "0") == "1"
    res = run_bass_kernel_spmd(
        nc, in_maps, core_ids=list(range(N_CORES)), trace=trace
    )
    LAST_RESULT = res

    out = np.empty((32, H, H, CO), np.float32)
    for c in range(N_CORES):
        o = res.results[c]["out_cm"]  # [256, 4, 12768]
        o = o.reshape(CO, NPC, H, WP)[:, :, :, :112]
        out[c * NPC : (c + 1) * NPC] = o.transpose(1, 2, 3, 0)
    return out


# revision 7
# speedup vs baseline: 1.0303x; 1.0032x over previous
"""BinaryConnect 3x3 SAME conv (NHWC, 32x112x112x128 -> 32x112x112x256) on 8 trn2 cores.

Strategy (data-parallel, 4 images per core):
  - Host: binarize kernel to +/-1 fp16 (exact), cast x to fp16, transpose to
    channel-major [cin, n, hp, wp] with 1-px zero halo, rows padded to 114 and
    one zero tail row (115 rows total), flattened per image.
  - Device: out[cout_half, fo] accumulates 9 matmuls (one per 3x3 tap) in PSUM:
    lhsT = wb_tap [cin=128, cout_half=128] (stationary),
    rhs  = x_flat[cin=128, fo + dh*114 + dw : +S] (moving, S=456),
    fp16 in, fp32 PSUM accumulate. Output written channel-major, un-transposed
    on host. Padded output columns (w=112,113) are garbage and stripped on host.
"""

import os

import numpy as np

import concourse.bass as bass
import concourse.mybir as mybir
import concourse.tile as tile
from concourse import bacc
from concourse.bass_utils import run_bass_kernel_spmd

N_CORES = 8
NPC = 4            # images per core
H = 112
WP = 114           # padded row width
HP = 115           # 1 top pad + 112 rows + 1 bottom pad + 1 zero tail row
XF = HP * WP       # 13110 flat padded-input positions per image
FO = H * WP        # 12768 flat padded-output positions per image
S = 456            # matmul free dim (4*114, divides FO; <=512 fp32 PSUM bank)
TS = FO // S       # 28 spatial tiles per image
CI = 128
CO = 256

_nc_cache = None
LAST_RESULT = None


def _build():
    nc = bacc.Bacc(
        "TRN2",
        target_bir_lowering=False,
        debug=False,
        num_devices=N_CORES,
    )
    x_d = nc.dram_tensor("xp", [CI, NPC, XF], mybir.dt.float16, kind="ExternalInput")
    w_d = nc.dram_tensor("wt", [CI, 9 * CO], mybir.dt.float16, kind="ExternalInput")
    o_d = nc.dram_tensor(
        "out_cm", [CO, NPC, FO], mybir.dt.float32, kind="ExternalOutput"
    )
    offs = [dh * WP + dw for dh in range(3) for dw in range(3)]
    with tile.TileContext(nc) as tc:
        with (
            tc.tile_pool(name="xpool", bufs=1) as xpool,
            tc.tile_pool(name="wpool", bufs=1) as wpool,
            tc.tile_pool(name="psum", bufs=8, space=bass.MemorySpace.PSUM) as psum,
            tc.tile_pool(name="opool", bufs=12) as opool,
        ):
            wt_s = wpool.tile([CI, 9 * CO], mybir.dt.float16)
            nc.sync.dma_start(wt_s[:], w_d[:, :])
            # Small first chunk of image 0 (rows 0-7) so the first real
            # matmul group (st=0) gates on ~200KB instead of a full band.
            XA = 8 * WP
            xa = xpool.tile([CI, XA], mybir.dt.float16, tag="xa", name="xa")
            nc.sync.dma_start(xa[:], x_d[:, 0, 0:XA])
            # Band-split the input so the first matmuls only gate on ~880KB:
            # 4 bands of 28 output rows per image; each band loads 31 input
            # rows (halo) = 3534 flat positions.
            BROWS = 28            # output rows per band
            NB = H // BROWS       # 4 bands per image
            BS = BROWS * WP       # 3192 output flat positions per band
            BIN = (BROWS + 3) * WP  # 3534 input flat positions incl. halo
            TSB = BS // S         # 7 spatial tiles per band
            # PE warmup: ~12 throwaway matmuls on the weight tile (arrives in
            # ~2us) so the HAM clock-gate is at 8/8 before the real stream and
            # the PE isn't idle-waiting on the first input band.
            wu = psum.tile([128, S], mybir.dt.float32, name="ps")
            for _ in range(12):
                nc.tensor.matmul(
                    wu[:], wt_s[:, 0:128], wt_s[:, 0:S], start=True, stop=True
                )
            xs = {}
            for n in range(NPC):
                for b in range(NB):
                    xt = xpool.tile(
                        [CI, BIN], mybir.dt.float16, tag=f"x{n}_{b}", name=f"x{n}_{b}"
                    )
                    nc.sync.dma_start(xt[:], x_d[:, n, b * BS : b * BS + BIN])
                    xs[n, b] = xt
            for n in range(NPC):
                for b in range(NB):
                    for st in range(TSB):
                        s0 = b * BS + st * S   # image-relative output offset
                        r0 = st * S            # band-relative offset
                        xsrc = xa if (n, b, st) == (0, 0, 0) else xs[n, b]
                        for half in range(2):
                            ps = psum.tile([128, S], mybir.dt.float32, name="ps")
                            for t in range(9):
                                w0 = t * CO + half * 128
                                nc.tensor.matmul(
                                    ps[:],
                                    wt_s[:, w0 : w0 + 128],
                                    xsrc[:, r0 + offs[t] : r0 + offs[t] + S],
                                    start=(t == 0),
                                    stop=(t == 8),
                                )
                            ot = opool.tile([128, S], mybir.dt.float32, name="ot")
                            nc.vector.tensor_copy(ot[:], ps[:])
                            # ACT's HWDGE ring — keeps output DMAs off the
                            # sync ring so they don't queue behind input DMAs.
                            nc.scalar.dma_start(
                                o_d[half * 128 : half * 128 + 128, n, s0 : s0 + S],
                                ot[:],
                            )
    nc.compile()
    return nc


def _get_nc():
    global _nc_cache
    if _nc_cache is None:
        _nc_cache = _build()
    return _nc_cache


def kernel(x, kernel):
    global LAST_RESULT
    x = np.asarray(x)
    k = np.asarray(kernel)

    wb = np.where(k >= 0, np.float16(1), np.float16(-1))  # [3,3,128,256]
    wt = np.ascontiguousarray(wb.transpose(2, 0, 1, 3).reshape(CI, 9 * CO))

    x16 = x.astype(np.float16)  # [32,112,112,128]
    in_maps = []
    for c in range(N_CORES):
        xp = np.zeros((CI, NPC, HP, WP), np.float16)
        xp[:, :, 1:113, 1:113] = x16[c * NPC : (c + 1) * NPC].transpose(3, 0, 1, 2)
        in_maps.append({"xp": xp.reshape(CI, NPC, XF), "wt": wt})

    nc = _get_nc()
    trace = os.environ.get("BCONV_TRACE", "0") == "1"
    res = run_bass_kernel_spmd(
        nc, in_maps, core_ids=list(range(N_CORES)), trace=trace
    )
    LAST_RESULT = res

    out = np.empty((32, H, H, CO), np.float32)
    for c in range(N_CORES):
        o = res.results[c]["out_cm"]  # [256, 4, 12768]
        o = o.reshape(CO, NPC, H, WP)[:, :, :, :112]
        out[c * NPC : (c + 1) * NPC] = o.transpose(1, 2, 3, 0)
    return out


# revision 10
# speedup vs baseline: 1.0322x; 1.0018x over previous
"""BinaryConnect 3x3 SAME conv (NHWC, 32x112x112x128 -> 32x112x112x256) on 8 trn2 cores.

Strategy (data-parallel, 4 images per core):
  - Host: binarize kernel to +/-1 fp16 (exact), cast x to fp16, transpose to
    channel-major [cin, n, hp, wp] with 1-px zero halo, rows padded to 114 and
    one zero tail row (115 rows total), flattened per image.
  - Device: out[cout_half, fo] accumulates 9 matmuls (one per 3x3 tap) in PSUM:
    lhsT = wb_tap [cin=128, cout_half=128] (stationary),
    rhs  = x_flat[cin=128, fo + dh*114 + dw : +S] (moving, S=456),
    fp16 in, fp32 PSUM accumulate. Output written channel-major, un-transposed
    on host. Padded output columns (w=112,113) are garbage and stripped on host.
"""

import os

import numpy as np

import concourse.bass as bass
import concourse.mybir as mybir
import concourse.tile as tile
from concourse import bacc
from concourse.bass_utils import run_bass_kernel_spmd

N_CORES = 8
NPC = 4            # images per core
H = 112
WP = 114           # padded row width
HP = 115           # 1 top pad + 112 rows + 1 bottom pad + 1 zero tail row
XF = HP * WP       # 13110 flat padded-input positions per image
FO = H * WP        # 12768 flat padded-output positions per image
S = 456            # matmul free dim (4*114, divides FO; <=512 fp32 PSUM bank)
TS = FO // S       # 28 spatial tiles per image
CI = 128
CO = 256

_nc_cache = None
LAST_RESULT = None


def _build():
    nc = bacc.Bacc(
        "TRN2",
        target_bir_lowering=False,
        debug=False,
        num_devices=N_CORES,
    )
    x_d = nc.dram_tensor("xp", [CI, NPC, XF], mybir.dt.float16, kind="ExternalInput")
    w_d = nc.dram_tensor("wt", [CI, 9 * CO], mybir.dt.float16, kind="ExternalInput")
    o_d = nc.dram_tensor(
        "out_cm", [CO, NPC, FO], mybir.dt.float32, kind="ExternalOutput"
    )
    offs = [dh * WP + dw for dh in range(3) for dw in range(3)]
    with tile.TileContext(nc) as tc:
        with (
            tc.tile_pool(name="xpool", bufs=1) as xpool,
            tc.tile_pool(name="wpool", bufs=1) as wpool,
            tc.tile_pool(name="psum", bufs=8, space=bass.MemorySpace.PSUM) as psum,
            tc.tile_pool(name="opool", bufs=12) as opool,
        ):
            # Tiny dedicated warmup-weight tile: first DMA in the queue, so PE
            # warmup can start as early as possible.
            wta = wpool.tile([CI, S], mybir.dt.float16, tag="wta", name="wta")
            nc.sync.dma_start(wta[:], w_d[:, 0:S])
            wt_s = wpool.tile([CI, 9 * CO], mybir.dt.float16)
            nc.sync.dma_start(wt_s[:], w_d[:, :])
            # Small first chunk of image 0 (rows 0-7) so the first real
            # matmul group (st=0) gates on ~200KB instead of a full band.
            XA = 8 * WP
            xa = xpool.tile([CI, XA], mybir.dt.float16, tag="xa", name="xa")
            nc.sync.dma_start(xa[:], x_d[:, 0, 0:XA])
            # Band-split the input so the first matmuls only gate on ~880KB:
            # 4 bands of 28 output rows per image; each band loads 31 input
            # rows (halo) = 3534 flat positions.
            BROWS = 28            # output rows per band
            NB = H // BROWS       # 4 bands per image
            BS = BROWS * WP       # 3192 output flat positions per band
            BIN = (BROWS + 3) * WP  # 3534 input flat positions incl. halo
            TSB = BS // S         # 7 spatial tiles per band
            # PE warmup: ~12 throwaway matmuls on the weight tile (arrives in
            # ~2us) so the HAM clock-gate is at 8/8 before the real stream and
            # the PE isn't idle-waiting on the first input band.
            wu = psum.tile([128, S], mybir.dt.float32, name="ps")
            for _ in range(9):
                nc.tensor.matmul(
                    wu[:], wta[:, 0:128], wta[:, 0:S], start=True, stop=True
                )
            xs = {}
            for n in range(NPC):
                for b in range(NB):
                    xt = xpool.tile(
                        [CI, BIN], mybir.dt.float16, tag=f"x{n}_{b}", name=f"x{n}_{b}"
                    )
                    nc.sync.dma_start(xt[:], x_d[:, n, b * BS : b * BS + BIN])
                    xs[n, b] = xt
            for n in range(NPC):
                for b in range(NB):
                    for st in range(TSB):
                        s0 = b * BS + st * S   # image-relative output offset
                        r0 = st * S            # band-relative offset
                        xsrc = xa if (n, b, st) == (0, 0, 0) else xs[n, b]
                        for half in range(2):
                            ps = psum.tile([128, S], mybir.dt.float32, name="ps")
                            for t in range(9):
                                w0 = t * CO + half * 128
                                nc.tensor.matmul(
                                    ps[:],
                                    wt_s[:, w0 : w0 + 128],
                                    xsrc[:, r0 + offs[t] : r0 + offs[t] + S],
                                    start=(t == 0),
                                    stop=(t == 8),
                                )
                            ot = opool.tile([128, S], mybir.dt.float32, name="ot")
                            nc.vector.tensor_copy(ot[:], ps[:])
                            # ACT's HWDGE ring — keeps output DMAs off the
                            # sync ring so they don't queue behind input DMAs.
                            nc.scalar.dma_start(
                                o_d[half * 128 : half * 128 + 128, n, s0 : s0 + S],
                                ot[:],
                            )
    nc.compile()
    return nc


def _get_nc():
    global _nc_cache
    if _nc_cache is None:
        _nc_cache = _build()
    return _nc_cache


def kernel(x, kernel):
    global LAST_RESULT
    x = np.asarray(x)
    k = np.asarray(kernel)

    wb = np.where(k >= 0, np.float16(1), np.float16(-1))  # [3,3,128,256]
    wt = np.ascontiguousarray(wb.transpose(2, 0, 1, 3).reshape(CI, 9 * CO))

    x16 = x.astype(np.float16)  # [32,112,112,128]
    in_maps = []
    for c in range(N_CORES):
        xp = np.zeros((CI, NPC, HP, WP), np.float16)
        xp[:, :, 1:113, 1:113] = x16[c * NPC : (c + 1) * NPC].transpose(3, 0, 1, 2)
        in_maps.append({"xp": xp.reshape(CI, NPC, XF), "wt": wt})

    nc = _get_nc()
    trace = os.environ.get("BCONV_TRACE", "0") == "1"
    kwargs = {}
    if trace and os.environ.get("BCONV_TRACE_CORES", "") == "all":
        kwargs["trace_cores"] = list(range(N_CORES))
    res = run_bass_kernel_spmd(
        nc, in_maps, core_ids=list(range(N_CORES)), trace=trace, **kwargs
    )
    LAST_RESULT = res

    out = np.empty((32, H, H, CO), np.float32)
    for c in range(N_CORES):
        o = res.results[c]["out_cm"]  # [256, 4, 12768]
        o = o.reshape(CO, NPC, H, WP)[:, :, :, :112]
        out[c * NPC : (c + 1) * NPC] = o.transpose(1, 2, 3, 0)
    return out


# revision 11
# speedup vs baseline: 1.0407x; 1.0083x over previous
"""BinaryConnect 3x3 SAME conv (NHWC, 32x112x112x128 -> 32x112x112x256) on 8 trn2 cores.

Strategy (data-parallel, 4 images per core):
  - Host: binarize kernel to +/-1 fp16 (exact), cast x to fp16, transpose to
    channel-major [cin, n, hp, wp] with 1-px zero halo, rows padded to 114 and
    one zero tail row (115 rows total), flattened per image.
  - Device: out[cout_half, fo] accumulates 9 matmuls (one per 3x3 tap) in PSUM:
    lhsT = wb_tap [cin=128, cout_half=128] (stationary),
    rhs  = x_flat[cin=128, fo + dh*114 + dw : +S] (moving, S=456),
    fp16 in, fp32 PSUM accumulate. Output written channel-major, un-transposed
    on host. Padded output columns (w=112,113) are garbage and stripped on host.
"""

import os

import numpy as np

import concourse.bass as bass
import concourse.mybir as mybir
import concourse.tile as tile
from concourse import bacc
from concourse.bass_utils import run_bass_kernel_spmd

N_CORES = 8
NPC = 4            # images per core
H = 112
WP = 114           # padded row width
HP = 115           # 1 top pad + 112 rows + 1 bottom pad + 1 zero tail row
XF = HP * WP       # 13110 flat padded-input positions per image
FO = H * WP        # 12768 flat padded-output positions per image
S = 456            # matmul free dim (4*114, divides FO; <=512 fp32 PSUM bank)
TS = FO // S       # 28 spatial tiles per image
CI = 128
CO = 256

_nc_cache = None
LAST_RESULT = None


def _build():
    nc = bacc.Bacc(
        "TRN2",
        target_bir_lowering=False,
        debug=False,
        num_devices=N_CORES,
    )
    x_d = nc.dram_tensor("xp", [CI, NPC, XF], mybir.dt.float16, kind="ExternalInput")
    w_d = nc.dram_tensor("wt", [CI, 9 * CO], mybir.dt.float16, kind="ExternalInput")
    o_d = nc.dram_tensor(
        "out_cm", [CO, NPC, FO], mybir.dt.float32, kind="ExternalOutput"
    )
    offs = [dh * WP + dw for dh in range(3) for dw in range(3)]
    with tile.TileContext(nc) as tc:
        with (
            tc.tile_pool(name="xpool", bufs=1) as xpool,
            tc.tile_pool(name="wpool", bufs=1) as wpool,
            tc.tile_pool(name="psum", bufs=8, space=bass.MemorySpace.PSUM) as psum,
            tc.tile_pool(name="opool", bufs=12) as opool,
        ):
            # Warmup operand with no DMA dependency: memset, so the PE warmup
            # (HAM un-throttle) can start right after the framework preamble,
            # overlapping the input DMA latency.
            wta = wpool.tile([CI, S], mybir.dt.float16, tag="wta", name="wta")
            nc.gpsimd.memset(wta[:], 0.0)
            wt_s = wpool.tile([CI, 9 * CO], mybir.dt.float16)
            nc.sync.dma_start(wt_s[:], w_d[:, :])
            # Small first chunk of image 0 (rows 0-7) so the first real
            # matmul group (st=0) gates on ~200KB instead of a full band.
            XA = 8 * WP
            xa = xpool.tile([CI, XA], mybir.dt.float16, tag="xa", name="xa")
            nc.sync.dma_start(xa[:], x_d[:, 0, 0:XA])
            # Band-split the input so the first matmuls only gate on ~880KB:
            # 4 bands of 28 output rows per image; each band loads 31 input
            # rows (halo) = 3534 flat positions.
            BROWS = 28            # output rows per band
            NB = H // BROWS       # 4 bands per image
            BS = BROWS * WP       # 3192 output flat positions per band
            BIN = (BROWS + 3) * WP  # 3534 input flat positions incl. halo
            TSB = BS // S         # 7 spatial tiles per band
            # PE warmup: ~12 throwaway matmuls on the weight tile (arrives in
            # ~2us) so the HAM clock-gate is at 8/8 before the real stream and
            # the PE isn't idle-waiting on the first input band.
            wu = psum.tile([128, S], mybir.dt.float32, name="ps")
            for _ in range(9):
                nc.tensor.matmul(
                    wu[:], wta[:, 0:128], wta[:, 0:S], start=True, stop=True
                )
            xs = {}
            for n in range(NPC):
                for b in range(NB):
                    xt = xpool.tile(
                        [CI, BIN], mybir.dt.float16, tag=f"x{n}_{b}", name=f"x{n}_{b}"
                    )
                    nc.sync.dma_start(xt[:], x_d[:, n, b * BS : b * BS + BIN])
                    xs[n, b] = xt
            for n in range(NPC):
                for b in range(NB):
                    for st in range(TSB):
                        s0 = b * BS + st * S   # image-relative output offset
                        r0 = st * S            # band-relative offset
                        xsrc = xa if (n, b, st) == (0, 0, 0) else xs[n, b]
                        for half in range(2):
                            ps = psum.tile([128, S], mybir.dt.float32, name="ps")
                            for t in range(9):
                                w0 = t * CO + half * 128
                                nc.tensor.matmul(
                                    ps[:],
                                    wt_s[:, w0 : w0 + 128],
                                    xsrc[:, r0 + offs[t] : r0 + offs[t] + S],
                                    start=(t == 0),
                                    stop=(t == 8),
                                )
                            ot = opool.tile([128, S], mybir.dt.float32, name="ot")
                            nc.vector.tensor_copy(ot[:], ps[:])
                            # ACT's HWDGE ring — keeps output DMAs off the
                            # sync ring so they don't queue behind input DMAs.
                            nc.scalar.dma_start(
                                o_d[half * 128 : half * 128 + 128, n, s0 : s0 + S],
                                ot[:],
                            )
    nc.compile()
    return nc


def _get_nc():
    global _nc_cache
    if _nc_cache is None:
        _nc_cache = _build()
    return _nc_cache


def kernel(x, kernel):
    global LAST_RESULT
    x = np.asarray(x)
    k = np.asarray(kernel)

    wb = np.where(k >= 0, np.float16(1), np.float16(-1))  # [3,3,128,256]
    wt = np.ascontiguousarray(wb.transpose(2, 0, 1, 3).reshape(CI, 9 * CO))

    x16 = x.astype(np.float16)  # [32,112,112,128]
    in_maps = []
    for c in range(N_CORES):
        xp = np.zeros((CI, NPC, HP, WP), np.float16)
        xp[:, :, 1:113, 1:113] = x16[c * NPC : (c + 1) * NPC].transpose(3, 0, 1, 2)
        in_maps.append({"xp": xp.reshape(CI, NPC, XF), "wt": wt})

    nc = _get_nc()
    trace = os.environ.get("BCONV_TRACE", "0") == "1"
    kwargs = {}
    if trace and os.environ.get("BCONV_TRACE_CORES", "") == "all":
        kwargs["trace_cores"] = list(range(N_CORES))
    res = run_bass_kernel_spmd(
        nc, in_maps, core_ids=list(range(N_CORES)), trace=trace, **kwargs
    )
    LAST_RESULT = res

    out = np.empty((32, H, H, CO), np.float32)
    for c in range(N_CORES):
        o = res.results[c]["out_cm"]  # [256, 4, 12768]
        o = o.reshape(CO, NPC, H, WP)[:, :, :, :112]
        out[c * NPC : (c + 1) * NPC] = o.transpose(1, 2, 3, 0)
    return out


# revision 12
# speedup vs baseline: 1.0506x; 1.0095x over previous
"""BinaryConnect 3x3 SAME conv (NHWC, 32x112x112x128 -> 32x112x112x256) on 8 trn2 cores.

Strategy (data-parallel, 4 images per core):
  - Host: binarize kernel to +/-1 fp16 (exact), cast x to fp16, transpose to
    channel-major [cin, n, hp, wp] with a 1-px zero halo (115x114 rows incl.
    one zero tail row).
  - Device: for each output tile of 4 rows x 112 cols (one cout half), the
    conv is 9 accumulating matmuls (one per 3x3 tap):
      lhsT = wb[tap] [cin=128, cout_half=128]   (stationary),
      rhs  = x[cin=128, rows r0+dh : r0+dh+4, cols dw : dw+112] (2D-AP moving,
             N = 448), fp16 in, fp32 PSUM accumulate.
    Output is written channel-major [cout, n, h*112+w] and un-transposed on
    the host. PE warmup matmuls on a memset tile un-throttle the HAM clock
    gate while the first input band DMA is still in flight.
"""

import os

import numpy as np

import concourse.bass as bass
import concourse.mybir as mybir
import concourse.tile as tile
from concourse import bacc
from concourse.bass_utils import run_bass_kernel_spmd

N_CORES = 8
NPC = 4            # images per core
H = 112
WP = 114           # padded row width
HP = 115           # 1 top pad + 112 rows + 1 bottom pad + 1 zero tail row
CI = 128
CO = 256
TROWS = 4          # output rows per matmul tile
S = TROWS * H      # 448 matmul free dim (<=512 fp32 PSUM bank)
BROWS = 28         # output rows per input band
NB = H // BROWS    # 4 bands per image
BIN = BROWS + 3    # input rows per band incl. halo
TSB = BROWS // TROWS  # 7 tiles per band

_nc_cache = None
LAST_RESULT = None


def _build():
    nc = bacc.Bacc(
        "TRN2",
        target_bir_lowering=False,
        debug=False,
        num_devices=N_CORES,
    )
    x_d = nc.dram_tensor(
        "xp", [CI, NPC, HP, WP], mybir.dt.float16, kind="ExternalInput"
    )
    w_d = nc.dram_tensor("wt", [CI, 9 * CO], mybir.dt.float16, kind="ExternalInput")
    o_d = nc.dram_tensor(
        "out_cm", [CO, NPC, H * H], mybir.dt.float32, kind="ExternalOutput"
    )
    with tile.TileContext(nc) as tc:
        with (
            tc.tile_pool(name="xpool", bufs=1) as xpool,
            tc.tile_pool(name="wpool", bufs=1) as wpool,
            tc.tile_pool(name="psum", bufs=8, space=bass.MemorySpace.PSUM) as psum,
            tc.tile_pool(name="opool", bufs=12) as opool,
        ):
            # Warmup operand with no DMA dependency: memset, so the PE warmup
            # (HAM un-throttle) can start right after the framework preamble,
            # overlapping the input DMA latency.
            wta = wpool.tile([CI, S], mybir.dt.float16, tag="wta", name="wta")
            nc.gpsimd.memset(wta[:], 0.0)
            wt_s = wpool.tile([CI, 9 * CO], mybir.dt.float16)
            nc.sync.dma_start(wt_s[:], w_d[:, :])
            # Small first chunk of image 0 (rows 0-7) so the first real
            # matmul group (st=0) gates on ~230KB instead of a full band.
            xa = xpool.tile([CI, 8, WP], mybir.dt.float16, tag="xa", name="xa")
            nc.sync.dma_start(xa[:], x_d[:, 0, 0:8, :])
            # PE warmup: 9 throwaway matmuls to push the HAM activity window
            # to K=8/8 before the real stream begins.
            wu = psum.tile([128, S], mybir.dt.float32, name="ps")
            for _ in range(9):
                nc.tensor.matmul(
                    wu[:], wta[:, 0:128], wta[:, 0:S], start=True, stop=True
                )
            # Band-split the input (4 bands of 28 output rows per image, 31
            # input rows each incl. halo) so compute gates on ~900KB chunks.
            xs = {}
            for n in range(NPC):
                for b in range(NB):
                    xt = xpool.tile(
                        [CI, BIN, WP],
                        mybir.dt.float16,
                        tag=f"x{n}_{b}",
                        name=f"x{n}_{b}",
                    )
                    nc.sync.dma_start(
                        xt[:], x_d[:, n, b * BROWS : b * BROWS + BIN, :]
                    )
                    xs[n, b] = xt
            for n in range(NPC):
                for b in range(NB):
                    for st in range(TSB):
                        r0 = st * TROWS       # band-relative top output row
                        o0 = (b * BROWS + st * TROWS) * H
                        xsrc = xa if (n, b, st) == (0, 0, 0) else xs[n, b]
                        for half in range(2):
                            ps = psum.tile([128, S], mybir.dt.float32, name="ps")
                            t = 0
                            for dh in range(3):
                                for dw in range(3):
                                    w0 = t * CO + half * 128
                                    nc.tensor.matmul(
                                        ps[:],
                                        wt_s[:, w0 : w0 + 128],
                                        xsrc[
                                            :,
                                            r0 + dh : r0 + dh + TROWS,
                                            dw : dw + H,
                                        ],
                                        start=(t == 0),
                                        stop=(t == 8),
                                    )
                                    t += 1
                            ot = opool.tile([128, S], mybir.dt.float32, name="ot")
                            nc.vector.tensor_copy(ot[:], ps[:])
                            # ACT's HWDGE ring — keeps output DMAs off the
                            # sync ring so they don't queue behind input DMAs.
                            nc.scalar.dma_start(
                                o_d[half * 128 : half * 128 + 128, n, o0 : o0 + S],
                                ot[:],
                            )
    nc.compile()
    return nc


def _get_nc():
    global _nc_cache
    if _nc_cache is None:
        _nc_cache = _build()
    return _nc_cache


def kernel(x, kernel):
    global LAST_RESULT
    x = np.asarray(x)
    k = np.asarray(kernel)

    # wt[ci, tap*256 + co] = sign(kernel[dh, dw, ci, co]), tap = dh*3 + dw
    wb = np.where(k >= 0, np.float16(1), np.float16(-1))  # [3,3,128,256]
    wt = np.ascontiguousarray(wb.transpose(2, 0, 1, 3).reshape(CI, 9 * CO))

    x16 = x.astype(np.float16)  # [32,112,112,128]
    in_maps = []
    for c in range(N_CORES):
        xp = np.zeros((CI, NPC, HP, WP), np.float16)
        xp[:, :, 1:113, 1:113] = x16[c * NPC : (c + 1) * NPC].transpose(3, 0, 1, 2)
        in_maps.append({"xp": xp, "wt": wt})

    nc = _get_nc()
    trace = os.environ.get("BCONV_TRACE", "0") == "1"
    kwargs = {}
    if trace and os.environ.get("BCONV_TRACE_CORES", "") == "all":
        kwargs["trace_cores"] = list(range(N_CORES))
    res = run_bass_kernel_spmd(
        nc, in_maps, core_ids=list(range(N_CORES)), trace=trace, **kwargs
    )
    LAST_RESULT = res

    out = np.empty((32, H, H, CO), np.float32)
    for c in range(N_CORES):
        o = res.results[c]["out_cm"].reshape(CO, NPC, H, H)
        out[c * NPC : (c + 1) * NPC] = o.transpose(1, 2, 3, 0)
    return out
